# Initial kernel scaffold
#
"""Malvar demosaic on Trainium2 (Bass/Tile), 8-core data parallel.

Input  x: [8, 4, 512, 512] f32 packed Bayer (channel c=2r+s holds mosaic
          pixels at (2i+r, 2j+s)).
Output  : [8, 3, 1024, 1024] f32, channels (b, g, r).

Strategy (per core, one image):
  The 5x5/stride-2 Malvar convs on the 1024x1024 mosaic are equivalent to a
  3x3 conv over the packed 4-channel space producing 12 output planes (8
  stencils + 4 passthroughs), one per (out-channel, row-parity, col-parity).
  All 12 planes are computed on the TensorEngine as 6 accumulating matmuls
  per 21-row block:
      lhsT[s,dj] : [92, 126] constants; K = 23 rows (incl +-1 halo) x 4 ch
                   (k = 23c + t), M = 6 families x 21 rows (m = 21*fam + i),
                   family = (out-ch, row-parity); entries encode the
                   vertical taps di = t-1-i.
      rhs        : input tile [92, 514] sliced at column offset 1+dj
                   (zero halo columns give dj padding).
      psum[s]    : [126, 512], accumulated over dj in {-1,0,+1}.
  The two column-parity PSUM tiles are interleaved into full 1024-wide
  mosaic rows with strided copies (VectorE for s=0, ScalarE for s=1), then
  each family's rows DMA to HBM as contiguous 4KB lines.
"""
import numpy as np

H, W = 512, 512
N_CORES = 8
N_ROWS = 21            # output packed rows per block
K_ROWS = N_ROWS + 2    # input rows incl halo
K_PART = 4 * K_ROWS    # 92
M_PART = 6 * N_ROWS    # 126
N_BLOCKS = (H + N_ROWS - 1) // N_ROWS  # 25
WPAD = W + 2           # 514, zero halo columns at 0 and W+1

_G_AT_R = np.array([[0,0,-1,0,0],[0,0,2,0,0],[-1,2,4,2,-1],[0,0,2,0,0],[0,0,-1,0,0]], np.float32) / 8
_R_AT_G1 = np.array([[0,0,0.5,0,0],[0,-1,0,-1,0],[-1,4,5,4,-1],[0,-1,0,-1,0],[0,0,0.5,0,0]], np.float32) / 8
_R_AT_G2 = np.array([[0,0,-1,0,0],[0,-1,4,-1,0],[0.5,0,5,0,0.5],[0,-1,4,-1,0],[0,0,-1,0,0]], np.float32) / 8
_R_AT_B = np.array([[0,0,-1.5,0,0],[0,2,0,2,0],[-1.5,0,6,0,-1.5],[0,2,0,2,0],[0,0,-1.5,0,0]], np.float32) / 8

PLANES = {
    (0, 0, 0): ('conv', _R_AT_B),
    (0, 0, 1): ('conv', _R_AT_G2),
    (0, 1, 0): ('conv', _R_AT_G1),
    (0, 1, 1): ('id', 2),
    (1, 0, 0): ('conv', _G_AT_R),
    (1, 0, 1): ('id', 0),
    (1, 1, 0): ('id', 3),
    (1, 1, 1): ('conv', _G_AT_R),
    (2, 0, 0): ('id', 1),
    (2, 0, 1): ('conv', _R_AT_G1),
    (2, 1, 0): ('conv', _R_AT_G2),
    (2, 1, 1): ('conv', _R_AT_B),
}


def _packed_weights():
    out = {}
    for (ch, r, s), (kind, val) in PLANES.items():
        Wk = np.zeros((4, 3, 3), np.float32)
        if kind == 'id':
            Wk[val, 1, 1] = 1.0
        else:
            for u in range(-2, 3):
                for v in range(-2, 3):
                    w = val[u + 2, v + 2]
                    if w == 0:
                        continue
                    rc = (r + u) % 2
                    di = (r + u - rc) // 2
                    sc = (s + v) % 2
                    dj = (s + v - sc) // 2
                    Wk[2 * rc + sc, di + 1, dj + 1] += w
        out[(ch, r, s)] = Wk
    return out


def _lhsT_matrices():
    Wp = _packed_weights()
    mats = np.zeros((2, 3, K_PART, M_PART), np.float32)
    for (ch, r, s), Wk in Wp.items():
        fam = 2 * ch + r
        for c in range(4):
            for t in range(K_ROWS):
                for i_loc in range(N_ROWS):
                    di = t - 1 - i_loc
                    if abs(di) > 1:
                        continue
                    for dj in range(-1, 2):
                        w = Wk[c, di + 1, dj + 1]
                        if w != 0:
                            mats[s, dj + 1, K_ROWS * c + t, N_ROWS * fam + i_loc] = w
    return mats


_NC_CACHE = {}


def _build(mm_dtype_name):
    import concourse.bacc as bacc
    import concourse.mybir as mybir
    import concourse.tile as tile

    mm_dt = getattr(mybir.dt, mm_dtype_name)
    f32 = mybir.dt.float32

    nc = bacc.Bacc("TRN2")
    x = nc.dram_tensor("x", [4, H, W], f32, kind="ExternalInput")
    out = nc.dram_tensor("out", [3, 2 * H, 2 * W], f32, kind="ExternalOutput")

    mats = _lhsT_matrices()  # [2, 3, 92, 126]
    # weights laid out as one [92, 6*126] const: slot q = 3*s + (dj+1)
    wflat = np.concatenate([mats[s, d] for s in range(2) for d in range(3)], axis=1)
    if mm_dtype_name == "bfloat16":
        import ml_dtypes
        wdata = wflat.astype(ml_dtypes.bfloat16)
    else:
        wdata = wflat  # float32 bits; float32r is a bitcast view
    wtens = nc.inline_tensor(wdata.copy(), name="wconst")

    with tile.TileContext(nc) as tc:
        with (
            tc.tile_pool(name="wpool", bufs=1) as wpool,
            tc.tile_pool(name="inpool", bufs=3) as inpool,
            tc.tile_pool(name="psum", bufs=4, space="PSUM") as psum_pool,
            tc.tile_pool(name="outpool", bufs=3) as outpool,
        ):
            w_sb = wpool.tile([K_PART, 6 * M_PART], mm_dt)
            nc.sync.dma_start(out=w_sb[:], in_=wtens[:].bitcast(mm_dt))

            for b in range(N_BLOCKS):
                i0 = b * N_ROWS
                r_lo = i0 - 1              # first input row (may be -1)
                r_hi = i0 + N_ROWS + 1     # one past last (may exceed H)
                t_lo = max(0, -r_lo)       # first valid t
                t_hi = K_ROWS - max(0, r_hi - H)  # one past last valid t
                nrows = min(N_ROWS, H - i0)

                in_t = inpool.tile([K_PART, WPAD], f32)
                # zero halo columns (0 and W+1) for all 4 channel groups
                nc.vector.memset(in_t[:, 0:WPAD:W + 1], 0.0)
                # zero out-of-image halo rows
                if t_lo > 0:
                    for c in range(4):
                        nc.vector.memset(in_t[K_ROWS * c: K_ROWS * c + t_lo, 1:W + 1], 0.0)
                if t_hi < K_ROWS:
                    for c in range(4):
                        nc.vector.memset(
                            in_t[K_ROWS * c + t_hi: K_ROWS * (c + 1), 1:W + 1], 0.0)
                for c in range(4):
                    nc.sync.dma_start(
                        out=in_t[K_ROWS * c + t_lo: K_ROWS * c + t_hi, 1:W + 1],
                        in_=x[c, r_lo + t_lo: r_lo + t_hi, :],
                    )

                rhs = in_t[:].bitcast(mm_dt) if mm_dtype_name != "bfloat16" else None
                if rhs is None:
                    rhs = inpool.tile([K_PART, WPAD], mm_dt, tag="in_bf")
                    nc.vector.tensor_copy(out=rhs[:], in_=in_t[:])
                    rhs = rhs[:]

                ps = []
                for s in range(2):
                    p = psum_pool.tile([M_PART, W], f32)
                    for dj in (-1, 0, 1):
                        q = 3 * s + (dj + 1)
                        nc.tensor.matmul(
                            p[:],
                            w_sb[:, M_PART * q: M_PART * (q + 1)],
                            rhs[:, 1 + dj: 1 + dj + W],
                            start=(dj == -1),
                            stop=(dj == 1),
                        )
                    ps.append(p)

                o_t = outpool.tile([M_PART, 2 * W], f32)
                nc.vector.tensor_copy(out=o_t[:, 0:2 * W:2], in_=ps[0][:])
                nc.scalar.copy(out=o_t[:, 1:2 * W:2], in_=ps[1][:])

                for fam in range(6):
                    ch, r = fam // 2, fam % 2
                    nc.sync.dma_start(
                        out=out[ch, 2 * i0 + r: 2 * (i0 + nrows) + r: 2, :],
                        in_=o_t[N_ROWS * fam: N_ROWS * fam + nrows, :],
                    )
    return nc


def _get_nc(mm_dtype_name="float32r"):
    if mm_dtype_name not in _NC_CACHE:
        _NC_CACHE[mm_dtype_name] = _build(mm_dtype_name)
    return _NC_CACHE[mm_dtype_name]


def kernel(x: np.ndarray, mm_dtype_name: str = "float32r", **run_kwargs) -> np.ndarray:
    from concourse.bass_utils import run_bass_kernel_spmd

    x = np.ascontiguousarray(np.asarray(x), dtype=np.float32)
    assert x.shape == (N_CORES, 4, H, W), x.shape
    nc = _get_nc(mm_dtype_name)
    in_maps = [{"x": x[b]} for b in range(N_CORES)]
    res = run_bass_kernel_spmd(nc, in_maps, core_ids=list(range(N_CORES)), **run_kwargs)
    return np.stack([r["out"] for r in res.results], axis=0)


if __name__ == "__main__":
    x = np.random.rand(N_CORES, 4, H, W).astype(np.float32)
    y = kernel(x)
    print("out", y.shape, y.dtype, float(y.sum()))


# revision 10
# speedup vs baseline: 1.9294x; 1.9294x over previous
"""Malvar demosaic on Trainium2 (Bass/Tile), 8-core data parallel.

Input  x: [8, 4, 512, 512] f32 packed Bayer (channel c=2r+s holds mosaic
          pixels at (2i+r, 2j+s)).
Output  : [8, 3, 1024, 1024] f32, channels (b, g, r).

Strategy (per core, one image):
  The 5x5/stride-2 Malvar convs on the 1024x1024 mosaic are equivalent to a
  3x3 conv over the packed 4-channel space producing 12 output planes (8
  stencils + 4 passthroughs), one per (out-channel, row-parity, col-parity).
  All 12 planes are computed on the TensorEngine as 6 accumulating matmuls
  per 21-row block:
      lhsT[s,dj] : [92, 126] constants; K = 23 rows (incl +-1 halo) x 4 ch
                   (k = 23c + t), M = 6 families x 21 rows (m = 21*fam + i),
                   family = (out-ch, row-parity); entries encode the
                   vertical taps di = t-1-i.
      rhs        : input tile [92, 514] sliced at column offset 1+dj
                   (zero halo columns give dj padding).
      psum[s]    : [126, 512], accumulated over dj in {-1,0,+1}.
  The two column-parity PSUM tiles are interleaved into full 1024-wide
  mosaic rows with strided copies (VectorE for s=0, ScalarE for s=1), then
  each family's rows DMA to HBM as contiguous 4KB lines.
"""
import numpy as np

H, W = 512, 512
N_CORES = 8
N_ROWS = 21            # output packed rows per block
K_ROWS = N_ROWS + 2    # input rows incl halo
K_PART = 4 * K_ROWS    # 92
M_PART = 6 * N_ROWS    # 126
N_BLOCKS = (H + N_ROWS - 1) // N_ROWS  # 25
WPAD = W + 2           # 514, zero halo columns at 0 and W+1

_G_AT_R = np.array([[0,0,-1,0,0],[0,0,2,0,0],[-1,2,4,2,-1],[0,0,2,0,0],[0,0,-1,0,0]], np.float32) / 8
_R_AT_G1 = np.array([[0,0,0.5,0,0],[0,-1,0,-1,0],[-1,4,5,4,-1],[0,-1,0,-1,0],[0,0,0.5,0,0]], np.float32) / 8
_R_AT_G2 = np.array([[0,0,-1,0,0],[0,-1,4,-1,0],[0.5,0,5,0,0.5],[0,-1,4,-1,0],[0,0,-1,0,0]], np.float32) / 8
_R_AT_B = np.array([[0,0,-1.5,0,0],[0,2,0,2,0],[-1.5,0,6,0,-1.5],[0,2,0,2,0],[0,0,-1.5,0,0]], np.float32) / 8

PLANES = {
    (0, 0, 0): ('conv', _R_AT_B),
    (0, 0, 1): ('conv', _R_AT_G2),
    (0, 1, 0): ('conv', _R_AT_G1),
    (0, 1, 1): ('id', 2),
    (1, 0, 0): ('conv', _G_AT_R),
    (1, 0, 1): ('id', 0),
    (1, 1, 0): ('id', 3),
    (1, 1, 1): ('conv', _G_AT_R),
    (2, 0, 0): ('id', 1),
    (2, 0, 1): ('conv', _R_AT_G1),
    (2, 1, 0): ('conv', _R_AT_G2),
    (2, 1, 1): ('conv', _R_AT_B),
}


def _packed_weights():
    out = {}
    for (ch, r, s), (kind, val) in PLANES.items():
        Wk = np.zeros((4, 3, 3), np.float32)
        if kind == 'id':
            Wk[val, 1, 1] = 1.0
        else:
            for u in range(-2, 3):
                for v in range(-2, 3):
                    w = val[u + 2, v + 2]
                    if w == 0:
                        continue
                    rc = (r + u) % 2
                    di = (r + u - rc) // 2
                    sc = (s + v) % 2
                    dj = (s + v - sc) // 2
                    Wk[2 * rc + sc, di + 1, dj + 1] += w
        out[(ch, r, s)] = Wk
    return out


def _lhsT_matrices():
    Wp = _packed_weights()
    mats = np.zeros((2, 3, K_PART, M_PART), np.float32)
    for (ch, r, s), Wk in Wp.items():
        fam = 2 * ch + r
        for c in range(4):
            for t in range(K_ROWS):
                for i_loc in range(N_ROWS):
                    di = t - 1 - i_loc
                    if abs(di) > 1:
                        continue
                    for dj in range(-1, 2):
                        w = Wk[c, di + 1, dj + 1]
                        if w != 0:
                            mats[s, dj + 1, K_ROWS * c + t, N_ROWS * fam + i_loc] = w
    return mats


_NC_CACHE = {}


def _build(mm_dtype_name, loop_iters=1):
    import contextlib

    import concourse.bacc as bacc
    import concourse.mybir as mybir
    import concourse.tile as tile

    mm_dt = getattr(mybir.dt, mm_dtype_name)
    f32 = mybir.dt.float32

    nc = bacc.Bacc("TRN2")
    x = nc.dram_tensor("x", [4, H, W], f32, kind="ExternalInput")
    out = nc.dram_tensor("out", [3, 2 * H, 2 * W], f32, kind="ExternalOutput")

    mats = _lhsT_matrices()  # [2, 3, 92, 126]
    # weights laid out as one [92, 6*126] const: slot q = 3*s + (dj+1)
    wflat = np.concatenate([mats[s, d] for s in range(2) for d in range(3)], axis=1)
    wtens = nc.inline_tensor(wflat.copy(), name="wconst")
    cast = mm_dtype_name != "float32"
    dma_in = nc.gpsimd if cast else nc.sync

    with tile.TileContext(nc) as tc:
        with (
            tc.tile_pool(name="wpool", bufs=1) as wpool,
            tc.tile_pool(name="inpool", bufs=3) as inpool,
            tc.tile_pool(name="psum", bufs=4, space="PSUM") as psum_pool,
            tc.tile_pool(name="outpool", bufs=3) as outpool,
        ):
            w_sb = wpool.tile([K_PART, 6 * M_PART], mm_dt)
            dma_in.dma_start(out=w_sb[:], in_=wtens[:])

            loop_cm = tc.For_i(0, loop_iters, 1) if loop_iters > 1 else contextlib.nullcontext()
            with loop_cm:
              for b in range(N_BLOCKS):
                i0 = b * N_ROWS
                r_lo = i0 - 1              # first input row (may be -1)
                r_hi = i0 + N_ROWS + 1     # one past last (may exceed H)
                t_lo = max(0, -r_lo)       # first valid t
                t_hi = K_ROWS - max(0, r_hi - H)  # one past last valid t
                nrows = min(N_ROWS, H - i0)

                in_t = inpool.tile([K_PART, WPAD], mm_dt)
                # memset can't encode float32r; write zeros through an f32 view
                mset = (lambda a: a.bitcast(f32)) if mm_dtype_name == "float32r" else (lambda a: a)
                if t_lo > 0 or t_hi < K_ROWS:
                    # edge block: zero whole tile (covers halo rows + columns)
                    nc.vector.memset(mset(in_t[:]), 0.0)
                else:
                    # zero halo columns (0 and W+1) for all 4 channel groups
                    nc.vector.memset(mset(in_t[:, 0:WPAD:W + 1]), 0.0)
                # SWDGE cast DMA only handles 2D APs correctly: one per channel
                for c in range(4):
                    dma_in.dma_start(
                        out=in_t[K_ROWS * c + t_lo: K_ROWS * c + t_hi, 1:W + 1],
                        in_=x[c, r_lo + t_lo: r_lo + t_hi, :],
                    )

                rhs = in_t[:]
                ps = []
                for s in range(2):
                    p = psum_pool.tile([M_PART, W], f32)
                    for dj in (-1, 0, 1):
                        q = 3 * s + (dj + 1)
                        nc.tensor.matmul(
                            p[:],
                            w_sb[:, M_PART * q: M_PART * (q + 1)],
                            rhs[:, 1 + dj: 1 + dj + W],
                            start=(dj == -1),
                            stop=(dj == 1),
                        )
                    ps.append(p)

                o_t = outpool.tile([M_PART, 2 * W], f32)
                nc.vector.tensor_copy(out=o_t[:, 0:2 * W:2], in_=ps[0][:])
                nc.scalar.copy(out=o_t[:, 1:2 * W:2], in_=ps[1][:])

                for fam in range(6):
                    ch, r = fam // 2, fam % 2
                    nc.sync.dma_start(
                        out=out[ch, 2 * i0 + r: 2 * (i0 + nrows): 2, :],
                        in_=o_t[N_ROWS * fam: N_ROWS * fam + nrows, :],
                    )
    nc.compile()
    return nc


def _get_nc(mm_dtype_name="float32r", loop_iters=1):
    key = (mm_dtype_name, loop_iters)
    if key not in _NC_CACHE:
        _NC_CACHE[key] = _build(mm_dtype_name, loop_iters)
    return _NC_CACHE[key]


def kernel(x: np.ndarray, mm_dtype_name: str = "float32r", **run_kwargs) -> np.ndarray:
    from concourse.bass_utils import run_bass_kernel_spmd

    x = np.ascontiguousarray(np.asarray(x), dtype=np.float32)
    assert x.shape == (N_CORES, 4, H, W), x.shape
    nc = _get_nc(mm_dtype_name)
    in_maps = [{"x": x[b]} for b in range(N_CORES)]
    res = run_bass_kernel_spmd(nc, in_maps, core_ids=list(range(N_CORES)), **run_kwargs)
    return np.stack([r["out"] for r in res.results], axis=0)


if __name__ == "__main__":
    x = np.random.rand(N_CORES, 4, H, W).astype(np.float32)
    y = kernel(x)
    print("out", y.shape, y.dtype, float(y.sum()))
